# revision 67
# baseline (speedup 1.0000x reference)
"""YOLOv1 loss kernel for Trainium2 (8 NeuronCores, data-parallel over batch).

Loss splits exactly into
    total = sum_obj [coor2 + es1 + 2*es2 + u1*(5*dcoor' + des) + cls]
          + sum_noobj 0.5*(c4^2 + c9^2)
with es_k = 0.5*e_k^2, e_k = conf_k - iou_k, u1 = [iou1 >= iou2],
coor' = coor/5, dcoor' = coor1' - coor2', des = es1 - es2.

Host packer: global obj/noobj partition (cells rebalanced across all 8
cores so every core gets identical W), constant-channel scaling (x3.5
halves), fp16 for the box pipeline, fp8(e4m3) for cls + noobj conf.

Channel layout per obj cell (cells along columns, 128 partitions):
  A = 3.5*[pw1,pw2,wg,l7 | ph1,ph2,hg,l8]   xo cols  0: 8W  (o-axis=4)
  B =     [px1,px2,xg,l5 | py1,py2,yg,l6]   xo cols 8W:16W
  PF =    [c1,c2]                           xo cols 16W:18W
  xc =    [pc1,lc1,...,pc7,lc7] fp8         14W (separate tensor)
  xn =    noobj [c4|c9] fp8                 2WN
Strided o-views give one-instruction fusions:
  x1 = Bv[:, :, 0:3] - Av[:, :, 0:3]   (pred corners + ground corner)
  dfa = Bv[:, :, 0:2] - Bv[:, :, 2:4]  (coor center diffs, both boxes)
  sd  = sqAv[:,:,0:2] - sqAv[:,:,2:4]  (coor sqrt diffs)
  ar  = A[0:3W]*A[4W:7W]               (areas a1,a2,ag)

Engines: DVE runs the serial box chain; ACT does sqrt/rsqrt/squares with
accum_out (alpha, beta, 5*coor2, noobj); GPSIMD does the cls diff +
square-accumulate from fp8.  All input DMA rides one queue (sync) in
consumption order B,A,PF,xc,xn so the first compute fires ~2.5us earlier
than bandwidth-shared queues would allow.  Weights fold into free
scalars: sds scale 1/sqrt(3.5), da's STT scalar 5, cps Copy scale 5.

ACT tables: sqrt set warms during DMA fill; one switch to the rsqrt set
(dummy rsqrt after sqg) hides under the DVE corner math.

Padding cells (equalizing shards) use identical pred/label boxes with
conf=1: contribution is LUT roundoff only (~1e-5 each).
"""

import numpy as np

B = 16384
NCORES = 8
CELLS = 49
NFLAT = B * CELLS          # 802816 cells global
P = 128

SQRTH = float(np.sqrt(0.5))
SDS_SCALE = float(1.0 / np.sqrt(3.5))


def _pack_all(pred, labels):
    """-> (xo (NC,P,18W) f16, xc (NC,P,14W) u8(fp8), xn (NC,P,2WN) u8, W, WN)"""
    import ml_dtypes

    prd = np.asarray(pred, np.float32).reshape(B, 17, CELLS)
    lab = np.asarray(labels, np.float32).reshape(B, 17, CELLS)
    prdT = np.ascontiguousarray(prd.transpose(1, 0, 2)).reshape(17, NFLAT)
    labT = np.ascontiguousarray(lab.transpose(1, 0, 2)).reshape(17, NFLAT)
    objf = labT[4] == 1.0
    oi = np.flatnonzero(objf)
    ni = np.flatnonzero(~objf)
    NO = len(oi)
    NN = len(ni)
    W = -(-NO // (NCORES * P))
    W += W % 2                       # even W keeps 4B alignment for 2x DVE
    WN = -(-NN // (NCORES * P))
    WN += WN % 2
    NOp = NCORES * P * W
    NNp = NCORES * P * WN

    po = prdT[:, oi]
    lo = labT[:, oi]

    # A: 3.5*[pw1,pw2,wg,l7 | ph1,ph2,hg,l8] as fp8; ACT upconverts on-chip
    # while the B halves stream on the other two queues
    xa = np.empty((8, NOp), ml_dtypes.float8_e4m3fn)
    for j, (src, ch) in enumerate([(po, 2), (po, 7), (lo, 2), (lo, 7),
                                   (po, 3), (po, 8), (lo, 3), (lo, 8)]):
        xa[j, :NO] = 3.5 * src[ch]
    xa[:, NO:] = 1.75

    xo = np.empty((10, NOp), np.float16)
    # B: [px1,px2,xg,l5 | py1,py2,yg,l6]
    for j, (src, ch) in enumerate([(po, 0), (po, 5), (lo, 0), (lo, 5),
                                   (po, 1), (po, 6), (lo, 1), (lo, 6)]):
        xo[j, :NO] = src[ch]
    xo[8, :NO] = po[4]
    xo[9, :NO] = po[9]
    xo[0:8, NO:] = 0.5
    xo[8:10, NO:] = 1.0

    # cls channels: pred and label blocks, interleaved at COLUMN level
    # after sharding (even col = pc, odd col = lc of the same cell+channel);
    # the kernel computes the 128x128 self-Gram on the tensor engine, so
    # the diagonal carries sum(pc^2)+sum(lc^2) and the superdiagonal at
    # even columns carries sum(pc*lc)
    xcp_ = np.empty((7, NOp), np.float16)
    xcl_ = np.empty((7, NOp), np.float16)
    for c in range(7):
        xcp_[c, :NO] = po[10 + c]
        xcl_[c, :NO] = lo[10 + c]
    xcp_[:, NO:] = 0.5
    xcl_[:, NO:] = 0.5

    # pre-scaled by sqrt(0.5) so a plain square-sum yields 0.5*c^2
    xn = np.zeros((2, NNp), ml_dtypes.float8_e4m3fn)
    xn[0, :NN] = SQRTH * prdT[4][ni]
    xn[1, :NN] = SQRTH * prdT[9][ni]

    def shard(a, C, Wx):
        # (C, NC*P*Wx) -> (NC, P, C*Wx); cell k of a shard = p*Wx + j
        a = a.reshape(C, NCORES, P, Wx).transpose(1, 2, 0, 3)
        return np.ascontiguousarray(a).reshape(NCORES, P, C * Wx)

    nblk = -(-14 * W // 128)
    xcp = np.zeros((NCORES, P, nblk * 128), np.float16)
    xcp[:, :, 0:14 * W:2] = shard(xcp_, 7, W)
    xcp[:, :, 1:14 * W:2] = shard(xcl_, 7, W)

    return (shard(xa.view(np.uint8), 8, W), shard(xo, 10, W), xcp,
            shard(xn.view(np.uint8), 2, WN), W, WN)


def _act_rsqrt(nc, mybir, out, in_):
    """ScalarE Rsqrt via raw InstActivation (bass wrapper bans Rsqrt);
    1/union = rsqrt(union)^2, measured end-to-end error ~1e-5."""
    imm = lambda v: mybir.ImmediateValue(dtype=mybir.dt.float32, value=v)
    eng = nc.scalar
    inst = mybir.InstActivation(
        name=nc.get_next_instruction_name(),
        func=mybir.ActivationFunctionType.Rsqrt,
        ins=[eng.lower_ap(in_), imm(0.0), imm(1.0), imm(0.0)],
        outs=[eng.lower_ap(out)],
    )
    return eng.add_instruction(inst)


def _build_nc(W, WN):
    import concourse.bass as bass
    import concourse.mybir as mybir
    from concourse.tile import TileContext
    from concourse.alu_op_type import AluOpType as op

    CT = mybir.dt.float16
    F8 = mybir.dt.float8e4
    F32 = mybir.dt.float32
    SQ = mybir.ActivationFunctionType.Square
    SQRT = mybir.ActivationFunctionType.Sqrt
    COPY = mybir.ActivationFunctionType.Copy

    nc = bass.Bass()
    NB = -(-14 * W // 128)
    xa_in = nc.dram_tensor("xa", [P, 8 * W], F8, kind="ExternalInput")
    xo_in = nc.dram_tensor("xo", [P, 10 * W], CT, kind="ExternalInput")
    xc_in = nc.dram_tensor("xc", [P, NB * 128], CT, kind="ExternalInput")
    xn_in = nc.dram_tensor("xn", [P, 2 * WN], F8, kind="ExternalInput")
    acc_out = nc.dram_tensor("acc", [P, 8], F32, kind="ExternalOutput")
    gram_out = nc.dram_tensor("gram", [P, P], F32, kind="ExternalOutput")

    def v4(ap):    # [P,8W] -> [P,2,4,W] (axis, o, w)
        return ap.rearrange("p (a o w) -> p a o w", a=2, o=4)

    def v22(ap):   # [P,4W] -> [P,2,2,W]
        return ap.rearrange("p (a o w) -> p a o w", a=2, o=2)

    with TileContext(nc) as tc:
        with (
            tc.tile_pool(name="inp", bufs=1) as inpool,
            tc.tile_pool(name="mid", bufs=1) as mid,
            tc.tile_pool(name="accp", bufs=1) as accp,
            tc.psum_pool(name="psum", bufs=1) as psum,
            nc.allow_low_precision("fp16 loss pipeline, validated vs ref"),
        ):
            acc = accp.tile([P, 8], F32)
            nc.vector.memset(acc[:, 4:5], 0.0)
            nc.vector.memset(acc[:, 6:8], 0.0)

            xat = inpool.tile([P, 8 * W], F8)
            xot = inpool.tile([P, 10 * W], CT)
            xct = inpool.tile([P, NB * 128], CT)
            xnt = inpool.tile([P, 2 * WN], F8)
            # three queues, byte-balanced: fp8 A whole on sync, B halves on
            # scalar+gpsimd; xn/PF trail sync, the cls block trails gpsimd
            nc.sync.dma_start(out=xat[:], in_=xa_in[:])
            nc.scalar.dma_start(out=xot[:, 4 * W:8 * W],
                                in_=xo_in[:, 4 * W:8 * W])      # B hi
            nc.gpsimd.dma_start(out=xot[:, 0:4 * W],
                                in_=xo_in[:, 0:4 * W])          # B lo
            nc.sync.dma_start(out=xnt[:], in_=xn_in[:])
            nc.sync.dma_start(out=xot[:, 8 * W:10 * W],
                              in_=xo_in[:, 8 * W:10 * W])       # PF
            HB = NB // 2
            nc.gpsimd.dma_start(out=xct[:, 0:HB * 128],
                                in_=xc_in[:, 0:HB * 128])
            nc.gpsimd.dma_start(out=xct[:, HB * 128:NB * 128],
                                in_=xc_in[:, HB * 128:NB * 128])

            Bt = xot[:, 0:8 * W]
            PF = xot[:, 8 * W:10 * W]
            Bv = v4(Bt)

            # ACT: warm sqrt loads the table set while DMAs fill, then the
            # two fp8->fp16 upconvert pieces chase the xa transfers
            warm = accp.tile([P, 2], CT)
            nc.vector.memset(warm[:], 1.0)
            wo_ = accp.tile([P, 2], CT)
            nc.scalar.activation(out=wo_[:], in_=warm[:], func=SQRT)
            At_t = mid.tile([P, 8 * W], CT)
            nc.scalar.activation(out=At_t[:], in_=xat[:], func=COPY)
            At = At_t[:]
            Av = v4(At)

            # --- DVE stream (program order ~= execution order) ---
            dfa = mid.tile([P, 4 * W], CT)       # B-only: coor center diffs
            nc.vector.tensor_tensor(out=v22(dfa[:]), in0=Bv[:, :, 0:2],
                                    in1=Bv[:, :, 2:4], op=op.subtract)

            # ACT: one contiguous sqrt of the whole A block; dummy rsqrt
            # then pulls the single table switch early, hidden under DVE math
            sqA = mid.tile([P, 8 * W], CT)
            nc.scalar.activation(out=sqA[:], in_=At, func=SQRT)
            _act_rsqrt(nc, mybir, wo_[:], sqA[:, 0:2])

            x1 = mid.tile([P, 6 * W], CT)        # corners: centers -+ halves
            x1v = x1[:].rearrange("p (a o w) -> p a o w", a=2, o=3)
            nc.vector.tensor_tensor(out=x1v, in0=Bv[:, :, 0:3],
                                    in1=Av[:, :, 0:3], op=op.subtract)
            x2 = mid.tile([P, 6 * W], CT)
            x2v = x2[:].rearrange("p (a o w) -> p a o w", a=2, o=3)
            nc.vector.tensor_tensor(out=x2v, in0=Bv[:, :, 0:3],
                                    in1=Av[:, :, 0:3], op=op.add)
            imax = mid.tile([P, 4 * W], CT)
            nc.vector.tensor_tensor(out=v22(imax[:]), in0=x1v[:, :, 0:2],
                                    in1=x1v[:, :, 2:3].broadcast_to([P, 2, 2, W]),
                                    op=op.max)
            imin = mid.tile([P, 4 * W], CT)
            nc.vector.tensor_tensor(out=v22(imin[:]), in0=x2v[:, :, 0:2],
                                    in1=x2v[:, :, 2:3].broadcast_to([P, 2, 2, W]),
                                    op=op.min)
            dd = mid.tile([P, 4 * W], CT)
            nc.vector.tensor_tensor(out=dd[:], in0=imin[:], in1=imax[:],
                                    op=op.subtract)
            dr = mid.tile([P, 4 * W], CT)
            nc.vector.tensor_scalar(out=dr[:], in0=dd[:], scalar1=0.0,
                                    scalar2=0.5, op0=op.max, op1=op.mult)
            inter = mid.tile([P, 2 * W], CT)
            nc.vector.tensor_tensor(out=inter[:], in0=dr[:, 0:2 * W],
                                    in1=dr[:, 2 * W:4 * W], op=op.mult)
            ar = mid.tile([P, 3 * W], CT)        # areas [a1,a2,ag]
            nc.vector.tensor_tensor(out=ar[:], in0=At[:, 0:3 * W],
                                    in1=At[:, 4 * W:7 * W], op=op.mult)
            arv = ar[:].rearrange("p (o w) -> p o w", o=3)
            uu = mid.tile([P, 2 * W], CT)
            nc.vector.tensor_tensor(out=uu[:].rearrange("p (o w) -> p o w", o=2),
                                    in0=arv[:, 0:2],
                                    in1=arv[:, 2:3].broadcast_to([P, 2, W]),
                                    op=op.add)
            un = mid.tile([P, 2 * W], CT)
            nc.vector.tensor_tensor(out=un[:], in0=uu[:], in1=inter[:],
                                    op=op.subtract)

            # ACT: rc fires the moment un lands
            rc = mid.tile([P, 2 * W], CT)
            _act_rsqrt(nc, mybir, rc[:], un[:])

            # DVE fills the rsqrt round-trip with the sqrt-diff (coor)
            sd = mid.tile([P, 4 * W], CT)
            sqAv = v4(sqA[:])
            nc.vector.tensor_tensor(out=v22(sd[:]), in0=sqAv[:, :, 0:2],
                                    in1=sqAv[:, :, 2:4], op=op.subtract)

            ih = mid.tile([P, 2 * W], CT)
            nc.vector.tensor_tensor(out=ih[:], in0=inter[:], in1=rc[:],
                                    op=op.mult)
            iou = mid.tile([P, 2 * W], CT)
            nc.vector.tensor_tensor(out=iou[:], in0=ih[:], in1=rc[:],
                                    op=op.mult)
            # zero bias depending on iou: pins flexible ACT work behind the
            # critical rsqrt round-trip in the static schedule
            zt = mid.tile([P, 1], CT)
            nc.vector.tensor_scalar(out=zt[:], in0=iou[:, 0:1], scalar1=0.0,
                                    scalar2=None, op0=op.mult)
            e = mid.tile([P, 2 * W], CT)
            nc.vector.tensor_tensor(out=e[:], in0=PF, in1=iou[:],
                                    op=op.subtract)
            u1c = mid.tile([P, W], CT)
            nc.vector.tensor_tensor(out=u1c[:], in0=iou[:, 0:W],
                                    in1=iou[:, W:2 * W], op=op.is_ge)

            # ACT: sds squares feed tq (quartered so rc can preempt fast);
            # dsqa rides DVE since ACT is the congested engine here
            sds = mid.tile([P, 4 * W], CT)
            for qq in range(4):
                nc.scalar.activation(out=sds[:, qq * W:(qq + 1) * W],
                                     in_=sd[:, qq * W:(qq + 1) * W],
                                     func=SQ, scale=SDS_SCALE)
            dsqa = mid.tile([P, 4 * W], CT)
            nc.vector.tensor_tensor(out=dsqa[:], in0=dfa[:], in1=dfa[:],
                                    op=op.mult)
            # es halves: DVE STT computes 0.5*e^2 into ce AND the alpha/beta
            # accumulators in one op each, keeping the merge tail engine-local
            ce = mid.tile([P, 4 * W], CT)        # [coorp'(2W) | es(2W)]
            nc.vector.scalar_tensor_tensor(out=ce[:, 2 * W:3 * W],
                                           in0=e[:, 0:W], scalar=0.5,
                                           op0=op.mult, in1=e[:, 0:W],
                                           op1=op.mult, accum_out=acc[:, 0:1])
            nc.vector.scalar_tensor_tensor(out=ce[:, 3 * W:4 * W],
                                           in0=e[:, W:2 * W], scalar=0.5,
                                           op0=op.mult, in1=e[:, W:2 * W],
                                           op1=op.mult, accum_out=acc[:, 1:2])

            tq = mid.tile([P, 4 * W], CT)
            nc.vector.tensor_tensor(out=tq[:], in0=dsqa[:], in1=sds[:],
                                    op=op.add)
            tqv = v22(tq[:])
            nc.vector.tensor_tensor(out=ce[:, 0:2 * W]
                                    .rearrange("p (o w) -> p o w", o=2),
                                    in0=tqv[:, 0], in1=tqv[:, 1], op=op.add)
            # [dcoor' | des] in one strided op, then fused merge + accum
            dde = mid.tile([P, 2 * W], CT)
            cev = ce[:].rearrange("p (a b w) -> p a b w", a=2, b=2)
            nc.vector.tensor_tensor(out=dde[:].rearrange("p (a w) -> p a w", a=2),
                                    in0=cev[:, :, 0], in1=cev[:, :, 1],
                                    op=op.subtract)
            da = mid.tile([P, W], CT)
            nc.vector.scalar_tensor_tensor(out=da[:], in0=dde[:, 0:W],
                                           scalar=5.0, in1=dde[:, W:2 * W],
                                           op0=op.mult, op1=op.add)
            sa = mid.tile([P, W], CT)
            nc.vector.scalar_tensor_tensor(out=sa[:], in0=da[:], scalar=0.0,
                                           op0=op.bypass, in1=u1c[:],
                                           op1=op.mult, accum_out=acc[:, 2:3])

            # 5*coor2 accum, right after coorp so it clears the ACT queue
            cps = mid.tile([P, W], CT)
            nc.scalar.activation(out=cps[:], in_=ce[:, W:2 * W],
                                 func=COPY, scale=5.0,
                                 accum_out=acc[:, 3:4])
            # noobj square-accum (xn pre-scaled by sqrt(.5) on host)
            ppsn = mid.tile([P, 2 * WN], CT)
            nc.scalar.activation(out=ppsn[:], in_=xnt[:], func=SQ,
                                 bias=zt[:, 0:1], accum_out=acc[:, 5:6])

            # PE: cls loss as an accumulated self-Gram of the column-
            # interleaved (pc,lc) block; host reads diag - 2*superdiag.
            # Emitted LAST so its PSUM->SBUF copy can't preempt the ACT
            # table switch / rc in the static schedule.
            G = psum.tile([P, P], F32)
            for j in range(NB):
                nc.tensor.matmul(G[:], xct[:, j * 128:(j + 1) * 128],
                                 xct[:, j * 128:(j + 1) * 128],
                                 start=(j == 0), stop=(j == NB - 1))
            Gs = mid.tile([P, P], F32)
            nc.scalar.activation(out=Gs[:], in_=G[:], func=COPY)
            nc.sync.dma_start(out=gram_out[:], in_=Gs[:])

            nc.sync.dma_start(out=acc_out[:], in_=acc[:])

    _split_multiwaits(nc, mybir)
    return nc


def _split_multiwaits(nc, mybir, max_waits=1):
    """This walrus build rejects instructions carrying more than one sem
    wait; hoist extra waits onto same-engine Drain instructions inserted
    immediately before the offender (semantically identical stall point)."""
    ctr = [0]
    for bb in nc.main_func.blocks:
        insts = bb.instructions
        out = []
        for ins in insts:
            si = ins.sync_info
            if si is not None and si.on_wait and len(si.on_wait) > max_waits:
                waits = list(si.on_wait)
                extra, keep = waits[:-max_waits], waits[-max_waits:]
                for k in range(0, len(extra), max_waits):
                    d = mybir.InstDrain(name=f"I-mw{ctr[0]}", ins=[], outs=[])
                    ctr[0] += 1
                    d.engine = ins.engine
                    d.sync_info = mybir.SyncInfo(on_wait=extra[k:k + max_waits],
                                                 on_update=[])
                    nc.register_instruction(d)
                    out.append(d)
                ins.sync_info = mybir.SyncInfo(on_wait=keep,
                                               on_update=list(si.on_update or []))
            out.append(ins)
        bb.instructions = out


_CACHED = {}


def kernel(pred, labels):
    from concourse.bass_utils import run_bass_kernel_spmd

    xa, xo, xc, xn, W, WN = _pack_all(pred, labels)
    key = (W, WN)
    if key not in _CACHED:
        _CACHED.clear()
        _CACHED[key] = _build_nc(W, WN)
    nc = _CACHED[key]

    in_maps = [{"xa": xa[i], "xo": xo[i], "xc": xc[i], "xn": xn[i]}
               for i in range(NCORES)]
    res = run_bass_kernel_spmd(nc, in_maps, core_ids=list(range(NCORES)))
    return _reduce_outputs([res.results[i] for i in range(NCORES)])


def _reduce_outputs(results):
    wts = np.array([1.0, 2.0, 1.0, 1.0, 0.0, 1.0, 0.0, 0.0], np.float64)
    ev = np.arange(0, P, 2)
    total = np.float64(0.0)
    for r in results:
        a = r["acc"].astype(np.float64)
        total += (a.sum(axis=0) * wts).sum()
        g = r["gram"].astype(np.float64)
        total += np.trace(g) - 2.0 * g[ev, ev + 1].sum()
    return np.asarray(total / B, dtype=np.float32)
